# revision 38
# baseline (speedup 1.0000x reference)
"""Trainium2 Bass kernel for CollisionDistanceEvaluator (segment_reduce).

Contract: kernel(**inputs) takes FULL inputs (trans [4096,3] f32,
quat [4096,4] f32, pc [4096,4096,3] f32) and returns the FULL output
[4096,1] f32, running the heavy per-point work on 8 NeuronCores
(pure data-parallel over the batch dim, 512 batches/core).

Math: reference rotates pc by inv(quat) (unit norm -> pure rotation R),
translates by -trans, tests an axis-aligned box, and takes the
per-batch masked mean of point norms. Host precomputes per batch:
    A[b] = R[b] / H[:,None]      (H = box half extents)
    o[b] = -(trans[b] + C) / H   (C = box center)
    g[b] = R[b]^T trans[b]       (|R p - t| = |p - g|, R orthogonal)
Device, per point p:
    u'_i  = A_i . p              <- TENSOR engine: 3 accumulated matmuls
                                    with diagonal [128,128] weights
                                    (per-batch scale), batch=partitions
    inside = max_i |u'_i + o_i| <= 1
    n2    = sum_i (p_i - g_i)^2
    out[b] = -10000 * sum(inside*sqrt(n2))/max(cnt,1)  (or +10000 if cnt==0)

Engine split per 128-batch tile (batch->partitions, 4096 points->free):
  PE   : 18 diag matmuls per 1024-pt round (u' planes, fp32 PSUM,
         2 banks per plane; ACT consumes both banks in one op)
  ACT  : Abs(u_i'+o_i) from PSUM; Square(x-g0) from SBUF;
         Sqrt(mask*n2) with free row-sum accumulation
  DVE  : max-combine, is_le mask with count on accum_out,
         (y-g1)^2 and (z-g2)^2 squares, n2 adds, mask*n2
"""

import numpy as np

import concourse.bass as bass
import concourse.bacc as bacc
import concourse.mybir as mybir
from concourse.tile import TileContext
from concourse.bass_utils import run_bass_kernel_spmd

def _ensure_ntff_hook():
    """Register the axon NTFF profile hook if the image's antenv lacks it."""
    import sys
    import types
    try:
        from antenv.axon_hooks import get_axon_ntff_profile_hook  # noqa
        return
    except ImportError:
        pass
    try:
        import antenv
        from trn_agent_boot.trn_boot import _ntff_profile_via_ctypes
        mod = types.ModuleType("antenv.axon_hooks")
        mod._hook = _ntff_profile_via_ctypes("/opt/axon/libaxon_pjrt.so")

        def set_axon_ntff_profile_hook(h):
            mod._hook = h

        def get_axon_ntff_profile_hook():
            return mod._hook

        mod.set_axon_ntff_profile_hook = set_axon_ntff_profile_hook
        mod.get_axon_ntff_profile_hook = get_axon_ntff_profile_hook
        sys.modules["antenv.axon_hooks"] = mod
        antenv.axon_hooks = mod
    except Exception:
        pass


_ensure_ntff_hook()

N_CORES = 8
B_FULL, N_PTS = 4096, 4096
B_CORE = B_FULL // N_CORES          # 512
N_TILES = B_CORE // 128             # 4
CHUNK = 512                          # PSUM bank: 512 f32
N_CHUNKS = N_PTS // CHUNK           # 8

DIST_THRESHOLD = 0.001
DIST_COEFF = 10000.0
BOX_CENTER = np.array([-0.001782, 1.005e-05, 0.0431621], dtype=np.float64)
HALF_EXT = np.array([
    0.204416 / 2 + DIST_THRESHOLD,
    0.0632517 / 2 + DIST_THRESHOLD,
    0.1381738 / 2 + DIST_THRESHOLD,
], dtype=np.float64)

_NC_CACHE = {}


def _build_bass():
    f16, f32 = mybir.dt.float16, mybir.dt.float32
    Alu = mybir.AluOpType
    Act = mybir.ActivationFunctionType

    nc = bacc.Bacc()
    xyz = nc.declare_dram_parameter(
        "xyz", [3, B_CORE, N_PTS], f16, isOutput=False)
    # per-batch scalars: cols 0-2 = o_i, cols 3-5 = -g_i
    coef = nc.declare_dram_parameter(
        "coef", [N_TILES, 128, 8], f32, isOutput=False)
    # diagonal PE weights: [tile, i, c] -> diag(A[batch, i, c])
    wd = nc.declare_dram_parameter(
        "wd", [N_TILES, 128, 9 * 128], f16, isOutput=False)
    out = nc.declare_dram_parameter("out", [B_CORE, 1], f32, isOutput=True)
    xyz_ap, coef_ap, wd_ap, out_ap = xyz[:], coef[:], wd[:], out[:]

    with TileContext(nc) as tc, \
            tc.tile_pool(name="data", bufs=2) as data, \
            tc.tile_pool(name="mask", bufs=2) as maskp, \
            tc.tile_pool(name="sq", bufs=2) as sqp, \
            tc.tile_pool(name="work", bufs=2) as work, \
            tc.tile_pool(name="wpool", bufs=2) as wpool, \
            tc.tile_pool(name="psum", bufs=1, space="PSUM") as psum, \
            tc.tile_pool(name="small", bufs=4) as small:
        # PE warm-up: junk matmuls during the initial DMA ramp keep the
        # HAM clock gate open so the first real matmuls run at 2.4 GHz
        wu = wpool.tile([128, 128], f16, tag="warm")
        nc.vector.memset(wu[:], 0.0)
        pwu = psum.tile([128, CHUNK], f32, tag="warm")
        for k in range(96):
            nc.tensor.matmul(pwu[:, :128], wu[:], wu[:, :],
                             start=True, stop=True)

        for t in range(N_TILES):
            r = slice(t * 128, (t + 1) * 128)
            # per-tile weights (one DMA), double-buffered for prefetch
            wt = wpool.tile([128, 9 * 128], f16, tag="wt")
            nc.sync.dma_start(out=wt[:], in_=wd_ap[t, :, :])
            ct = small.tile([128, 8], f32, tag="coef")
            nc.sync.dma_start(out=ct[:], in_=coef_ap[t, :, :])
            xt = data.tile([128, N_PTS], f16, tag="x")
            yt = data.tile([128, N_PTS], f16, tag="y")
            zt = data.tile([128, N_PTS], f16, tag="z")
            half = N_PTS // 2
            for hh in range(2):
                s = slice(hh * half, (hh + 1) * half)
                nc.sync.dma_start(out=zt[:, s], in_=xyz_ap[2, r, s])
                nc.sync.dma_start(out=xt[:, s], in_=xyz_ap[0, r, s])
                nc.sync.dma_start(out=yt[:, s], in_=xyz_ap[1, r, s])
            planes = (xt, yt, zt)

            # mask-plane tiles (full 4096 wide, filled chunkwise)
            a0 = maskp.tile([128, N_PTS], f16, tag="a0")
            a1 = maskp.tile([128, N_PTS], f16, tag="a1")
            a2 = maskp.tile([128, N_PTS], f16, tag="a2")

            def wsel(i, c, wt=wt):
                w = 3 * i + c
                return wt[:, w * 128:(w + 1) * 128]

            w0 = sqp.tile([128, N_PTS], f16, tag="w0")
            w1 = sqp.tile([128, N_PTS], f16, tag="w1")
            w2 = sqp.tile([128, N_PTS], f16, tag="w2")
            # rounds of 2 PSUM banks (1024 pts): one matmul writes <= 1
            # bank, but ACT reads across both in a single op.  Last
            # tile ends with two 512-pt rounds to shorten the serial
            # tail chain after the final matmul.
            rounds = [(k * 2 * CHUNK, 2 * CHUNK) for k in range(4)]
            if t == N_TILES - 1:
                rounds = rounds[:3] + [(6 * CHUNK, CHUNK),
                                       (7 * CHUNK, CHUNK)]
            for j, (c0, cw) in enumerate(rounds):
                cs = slice(c0, c0 + cw)
                u0 = psum.tile([128, 2 * CHUNK], f32, tag="u0")
                u1 = psum.tile([128, 2 * CHUNK], f32, tag="u1")
                u2 = psum.tile([128, 2 * CHUNK], f32, tag="u2")
                for i, ut in enumerate((u0, u1, u2)):
                    for c in range(3):
                        for h in range(cw // CHUNK):
                            hs = slice(h * CHUNK, (h + 1) * CHUNK)
                            ps = slice(c0 + h * CHUNK,
                                       c0 + (h + 1) * CHUNK)
                            nc.tensor.matmul(
                                ut[:, hs], wsel(i, c), planes[c][:, ps],
                                start=(c == 0), stop=(c == 2))
                # |u'+o| on ACT (fast PSUM reads, free bias add)
                nc.scalar.activation(
                    a0[:, cs], u0[:, :cw], Act.Abs, bias=ct[:, 0:1])
                nc.scalar.activation(
                    a1[:, cs], u1[:, :cw], Act.Abs, bias=ct[:, 1:2])
                nc.scalar.activation(
                    a2[:, cs], u2[:, :cw], Act.Abs, bias=ct[:, 2:3])
                # interleave ACT square pieces between ABS bursts so
                # the n2 adds never wait on a late x/y square
                if j < 2:
                    s = slice(j * half, (j + 1) * half)
                    nc.scalar.activation(
                        w0[:, s], xt[:, s], Act.Square, bias=ct[:, 3:4])
                elif j == 2:
                    s = slice(half, N_PTS)
                    nc.scalar.activation(
                        w1[:, s], yt[:, s], Act.Square, bias=ct[:, 4:5])

            # remaining squares on DVE: (y-g1)^2 first half, (z-g2)^2
            # halved so work starts as soon as the first DMAs land.
            for hh in range(2):
                s = slice(hh * half, (hh + 1) * half)
                nc.vector.tensor_scalar(
                    w2[:, s], zt[:, s], ct[:, 5:6], None, Alu.add)
                nc.vector.tensor_tensor(
                    w2[:, s], w2[:, s], w2[:, s], Alu.mult)
                if hh == 0:
                    nc.vector.tensor_scalar(
                        w1[:, s], yt[:, s], ct[:, 4:5], None, Alu.add)
                    nc.vector.tensor_tensor(
                        w1[:, s], w1[:, s], w1[:, s], Alu.mult)

            # per-round tail so it pipelines with this tile's matmuls:
            # mask = (max(a0,a1,a2) <= 1), count on accum_out;
            # n2 sum; mm = mask*n2; Sqrt accum -> per-round partials
            mx = work.tile([128, N_PTS], f16, tag="mx")
            mask = work.tile([128, N_PTS], f16, tag="mask")
            rcv = small.tile([128, 5], f32, tag="rcv")
            rsv = small.tile([128, 5], f32, tag="rsv")
            nr = len(rounds)
            for j, (c0, cw) in enumerate(rounds):
                cs = slice(c0, c0 + cw)
                nc.vector.tensor_tensor(
                    mx[:, cs], a0[:, cs], a1[:, cs], Alu.max)
                nc.vector.tensor_tensor(
                    mx[:, cs], mx[:, cs], a2[:, cs], Alu.max)
                nc.vector.tensor_scalar(
                    mask[:, cs], mx[:, cs], 1.0, 0.0, Alu.is_le, Alu.add,
                    accum_out=rcv[:, j:j + 1])
                nc.vector.tensor_tensor(
                    w0[:, cs], w0[:, cs], w1[:, cs], Alu.add)
                nc.vector.tensor_tensor(
                    w0[:, cs], w0[:, cs], w2[:, cs], Alu.add)
                nc.vector.tensor_tensor(
                    mx[:, cs], mask[:, cs], w0[:, cs], Alu.mult)
                nc.scalar.activation(
                    w1[:, cs], mx[:, cs], Act.Sqrt,
                    accum_out=rsv[:, j:j + 1])
            rc = small.tile([128, 1], f32, tag="rc")
            rs = small.tile([128, 1], f32, tag="rs")
            nc.vector.tensor_reduce(
                rc[:], rcv[:, 0:nr], mybir.AxisListType.X, Alu.add)
            nc.vector.tensor_reduce(
                rs[:], rsv[:, 0:nr], mybir.AxisListType.X, Alu.add)

            # out = (cnt==0)*10000 + (-10000*rs)/max(cnt,1)
            rc1 = small.tile([128, 1], f32, tag="rc1")
            nc.vector.tensor_scalar(rc1[:], rc[:], 1.0, None, Alu.max)
            inv = small.tile([128, 1], f32, tag="inv")
            nc.vector.reciprocal(inv[:], rc1[:])
            val = small.tile([128, 1], f32, tag="val")
            nc.vector.scalar_tensor_tensor(
                val[:], rs[:], -DIST_COEFF, inv[:], Alu.mult, Alu.mult)
            zer = small.tile([128, 1], f32, tag="zer")
            nc.vector.tensor_scalar(zer[:], rc[:], 0.0, None, Alu.is_le)
            ot = small.tile([128, 1], f32, tag="ot")
            nc.vector.scalar_tensor_tensor(
                ot[:], zer[:], DIST_COEFF, val[:], Alu.mult, Alu.add)
            nc.sync.dma_start(out=out_ap[r, :], in_=ot[:])
    nc.compile()
    return nc


def _get_nc():
    if "nc" not in _NC_CACHE:
        _NC_CACHE["nc"] = _build_bass()
    return _NC_CACHE["nc"]


def _host_coefficients(trans, quat):
    """Per-batch A [B,3,3], o [B,3], g [B,3] in f32 (computed in f64)."""
    q = np.asarray(quat, np.float64)
    t = np.asarray(trans, np.float64)
    B = q.shape[0]
    s = (q * q).sum(-1)
    qi = np.concatenate([-q[:, :3], q[:, 3:]], -1) / s[:, None]
    v, w = qi[:, :3], qi[:, 3]
    vv = v[:, :, None] * v[:, None, :]
    w2mv = w * w - (v * v).sum(-1)
    Vx = np.zeros((B, 3, 3))
    Vx[:, 0, 1] = -v[:, 2]
    Vx[:, 0, 2] = v[:, 1]
    Vx[:, 1, 0] = v[:, 2]
    Vx[:, 1, 2] = -v[:, 0]
    Vx[:, 2, 0] = -v[:, 1]
    Vx[:, 2, 1] = v[:, 0]
    R = (w2mv[:, None, None] * np.eye(3)
         + 2.0 * vv
         + 2.0 * w[:, None, None] * Vx)
    A = R / HALF_EXT[None, :, None]
    o = -(t + BOX_CENTER[None, :]) / HALF_EXT[None, :]
    g = np.einsum("bij,bj->bi", R.transpose(0, 2, 1), t)
    return A.astype(np.float32), o.astype(np.float32), g.astype(np.float32)


def _make_in_maps(trans, quat, pc):
    A, o, g = _host_coefficients(trans, quat)
    coef_full = np.concatenate(
        [o, -g, np.zeros((B_FULL, 2), np.float32)], axis=1)  # [B,8]
    # planar fp16 [3, B, N]
    pcT = np.ascontiguousarray(
        np.asarray(pc, np.float32).transpose(2, 0, 1)).astype(np.float16)
    # diagonal weights [tile, i, c] per core
    idx = np.arange(128)
    in_maps = []
    for cidx in range(N_CORES):
        bs, be = cidx * B_CORE, (cidx + 1) * B_CORE
        Ac = A[bs:be].reshape(N_TILES, 128, 3, 3)
        wdc = np.zeros((N_TILES, 3, 3, 128, 128), np.float16)
        wdc[:, :, :, idx, idx] = np.transpose(
            Ac, (0, 2, 3, 1)).astype(np.float16)
        in_maps.append({
            "xyz": np.ascontiguousarray(pcT[:, bs:be, :]),
            "coef": np.ascontiguousarray(
                coef_full[bs:be].reshape(N_TILES, 128, 8)),
            "wd": np.ascontiguousarray(
                np.transpose(wdc.reshape(N_TILES, 9, 128, 128),
                             (0, 2, 1, 3)).reshape(N_TILES, 128, 9 * 128)),
        })
    return in_maps


def run_spmd(trans, quat, pc, **spmd_kwargs):
    """Shard, run on 8 cores, gather. Returns (output, BassKernelResults)."""
    in_maps = _make_in_maps(trans, quat, pc)
    res = run_bass_kernel_spmd(
        _get_nc(), in_maps, list(range(N_CORES)), **spmd_kwargs)
    outs = [res.results[i]["out"] for i in range(N_CORES)]
    full = np.concatenate(outs, axis=0).astype(np.float32)
    return full, res


def kernel(trans, quat, pc):
    full, _ = run_spmd(trans, quat, pc)
    return full


# revision 39
# speedup vs baseline: 1.0135x; 1.0135x over previous
"""Trainium2 Bass kernel for CollisionDistanceEvaluator (segment_reduce).

Contract: kernel(**inputs) takes FULL inputs (trans [4096,3] f32,
quat [4096,4] f32, pc [4096,4096,3] f32) and returns the FULL output
[4096,1] f32, running the heavy per-point work on 8 NeuronCores
(pure data-parallel over the batch dim, 512 batches/core).

Math: reference rotates pc by inv(quat) (unit norm -> pure rotation R),
translates by -trans, tests an axis-aligned box, and takes the
per-batch masked mean of point norms. Host precomputes per batch:
    A[b] = R[b] / H[:,None]      (H = box half extents)
    o[b] = -(trans[b] + C) / H   (C = box center)
    g[b] = R[b]^T trans[b]       (|R p - t| = |p - g|, R orthogonal)
Device, per point p:
    u'_i  = A_i . p              <- TENSOR engine: 3 accumulated matmuls
                                    with diagonal [128,128] weights
                                    (per-batch scale), batch=partitions
    inside = max_i |u'_i + o_i| <= 1
    n2    = sum_i (p_i - g_i)^2
    out[b] = -10000 * sum(inside*sqrt(n2))/max(cnt,1)  (or +10000 if cnt==0)

Engine split per 128-batch tile (batch->partitions, 4096 points->free):
  PE   : 18 diag matmuls per 1024-pt round (u' planes, fp32 PSUM,
         2 banks per plane; ACT consumes both banks in one op)
  ACT  : Abs(u_i'+o_i) from PSUM; Square(x-g0) from SBUF;
         Sqrt(mask*n2) with free row-sum accumulation
  DVE  : max-combine, is_le mask with count on accum_out,
         (y-g1)^2 and (z-g2)^2 squares, n2 adds, mask*n2
"""

import numpy as np

import concourse.bass as bass
import concourse.bacc as bacc
import concourse.mybir as mybir
from concourse.tile import TileContext
from concourse.bass_utils import run_bass_kernel_spmd

def _ensure_ntff_hook():
    """Register the axon NTFF profile hook if the image's antenv lacks it."""
    import sys
    import types
    try:
        from antenv.axon_hooks import get_axon_ntff_profile_hook  # noqa
        return
    except ImportError:
        pass
    try:
        import antenv
        from trn_agent_boot.trn_boot import _ntff_profile_via_ctypes
        mod = types.ModuleType("antenv.axon_hooks")
        mod._hook = _ntff_profile_via_ctypes("/opt/axon/libaxon_pjrt.so")

        def set_axon_ntff_profile_hook(h):
            mod._hook = h

        def get_axon_ntff_profile_hook():
            return mod._hook

        mod.set_axon_ntff_profile_hook = set_axon_ntff_profile_hook
        mod.get_axon_ntff_profile_hook = get_axon_ntff_profile_hook
        sys.modules["antenv.axon_hooks"] = mod
        antenv.axon_hooks = mod
    except Exception:
        pass


_ensure_ntff_hook()

N_CORES = 8
B_FULL, N_PTS = 4096, 4096
B_CORE = B_FULL // N_CORES          # 512
N_TILES = B_CORE // 128             # 4
CHUNK = 512                          # PSUM bank: 512 f32
N_CHUNKS = N_PTS // CHUNK           # 8

DIST_THRESHOLD = 0.001
DIST_COEFF = 10000.0
BOX_CENTER = np.array([-0.001782, 1.005e-05, 0.0431621], dtype=np.float64)
HALF_EXT = np.array([
    0.204416 / 2 + DIST_THRESHOLD,
    0.0632517 / 2 + DIST_THRESHOLD,
    0.1381738 / 2 + DIST_THRESHOLD,
], dtype=np.float64)

_NC_CACHE = {}


def _build_bass():
    f16, f32 = mybir.dt.float16, mybir.dt.float32
    Alu = mybir.AluOpType
    Act = mybir.ActivationFunctionType

    nc = bacc.Bacc()
    xyz = nc.declare_dram_parameter(
        "xyz", [3, B_CORE, N_PTS], f16, isOutput=False)
    # per-batch scalars: cols 0-2 = o_i, cols 3-5 = -g_i
    coef = nc.declare_dram_parameter(
        "coef", [N_TILES, 128, 8], f32, isOutput=False)
    # diagonal PE weights: [tile, i, c] -> diag(A[batch, i, c])
    wd = nc.declare_dram_parameter(
        "wd", [N_TILES, 128, 9 * 128], f16, isOutput=False)
    out = nc.declare_dram_parameter("out", [B_CORE, 1], f32, isOutput=True)
    xyz_ap, coef_ap, wd_ap, out_ap = xyz[:], coef[:], wd[:], out[:]

    with TileContext(nc) as tc, \
            tc.tile_pool(name="data", bufs=2) as data, \
            tc.tile_pool(name="mask", bufs=2) as maskp, \
            tc.tile_pool(name="sq", bufs=2) as sqp, \
            tc.tile_pool(name="work", bufs=2) as work, \
            tc.tile_pool(name="wpool", bufs=2) as wpool, \
            tc.tile_pool(name="psum", bufs=1, space="PSUM") as psum, \
            tc.tile_pool(name="small", bufs=4) as small:
        # PE warm-up: junk matmuls during the initial DMA ramp keep the
        # HAM clock gate open so the first real matmuls run at 2.4 GHz
        wu = wpool.tile([128, 128], f16, tag="warm")
        nc.vector.memset(wu[:], 0.0)
        pwu = psum.tile([128, CHUNK], f32, tag="warm")
        for k in range(96):
            nc.tensor.matmul(pwu[:, :128], wu[:], wu[:, :],
                             start=True, stop=True)

        for t in range(N_TILES):
            r = slice(t * 128, (t + 1) * 128)
            # per-tile weights (one DMA), double-buffered for prefetch
            wt = wpool.tile([128, 9 * 128], f16, tag="wt")
            nc.sync.dma_start(out=wt[:], in_=wd_ap[t, :, :])
            ct = small.tile([128, 8], f32, tag="coef")
            nc.sync.dma_start(out=ct[:], in_=coef_ap[t, :, :])
            xt = data.tile([128, N_PTS], f16, tag="x")
            yt = data.tile([128, N_PTS], f16, tag="y")
            zt = data.tile([128, N_PTS], f16, tag="z")
            half = N_PTS // 2
            for hh in range(2):
                s = slice(hh * half, (hh + 1) * half)
                nc.sync.dma_start(out=zt[:, s], in_=xyz_ap[2, r, s])
                nc.sync.dma_start(out=xt[:, s], in_=xyz_ap[0, r, s])
                nc.sync.dma_start(out=yt[:, s], in_=xyz_ap[1, r, s])
            planes = (xt, yt, zt)

            # mask-plane tiles (full 4096 wide, filled chunkwise)
            a0 = maskp.tile([128, N_PTS], f16, tag="a0")
            a1 = maskp.tile([128, N_PTS], f16, tag="a1")
            a2 = maskp.tile([128, N_PTS], f16, tag="a2")

            def wsel(i, c, wt=wt):
                w = 3 * i + c
                return wt[:, w * 128:(w + 1) * 128]

            w0 = sqp.tile([128, N_PTS], f16, tag="w0")
            w1 = sqp.tile([128, N_PTS], f16, tag="w1")
            w2 = sqp.tile([128, N_PTS], f16, tag="w2")
            # rounds of 2 PSUM banks (1024 pts): one matmul writes <= 1
            # bank, but ACT reads across both in a single op
            for j in range(N_CHUNKS // 2):
                cs = slice(j * 2 * CHUNK, (j + 1) * 2 * CHUNK)
                u0 = psum.tile([128, 2 * CHUNK], f32, tag="u0")
                u1 = psum.tile([128, 2 * CHUNK], f32, tag="u1")
                u2 = psum.tile([128, 2 * CHUNK], f32, tag="u2")
                for i, ut in enumerate((u0, u1, u2)):
                    for c in range(3):
                        for h in range(2):
                            hs = slice(h * CHUNK, (h + 1) * CHUNK)
                            ps = slice((j * 2 + h) * CHUNK,
                                       (j * 2 + h + 1) * CHUNK)
                            nc.tensor.matmul(
                                ut[:, hs], wsel(i, c), planes[c][:, ps],
                                start=(c == 0), stop=(c == 2))
                # |u'+o| on ACT (fast PSUM reads, free bias add)
                nc.scalar.activation(
                    a0[:, cs], u0[:], Act.Abs, bias=ct[:, 0:1])
                nc.scalar.activation(
                    a1[:, cs], u1[:], Act.Abs, bias=ct[:, 1:2])
                nc.scalar.activation(
                    a2[:, cs], u2[:], Act.Abs, bias=ct[:, 2:3])
                # interleave ACT square pieces between ABS bursts so
                # the n2 adds never wait on a late x/y square
                if j < 2:
                    s = slice(j * half, (j + 1) * half)
                    nc.scalar.activation(
                        w0[:, s], xt[:, s], Act.Square, bias=ct[:, 3:4])
                elif j == 2:
                    s = slice(half, N_PTS)
                    nc.scalar.activation(
                        w1[:, s], yt[:, s], Act.Square, bias=ct[:, 4:5])

            # remaining squares on DVE: (y-g1)^2 first half, (z-g2)^2
            # halved so work starts as soon as the first DMAs land.
            for hh in range(2):
                s = slice(hh * half, (hh + 1) * half)
                nc.vector.tensor_scalar(
                    w2[:, s], zt[:, s], ct[:, 5:6], None, Alu.add)
                nc.vector.tensor_tensor(
                    w2[:, s], w2[:, s], w2[:, s], Alu.mult)
                if hh == 0:
                    nc.vector.tensor_scalar(
                        w1[:, s], yt[:, s], ct[:, 4:5], None, Alu.add)
                    nc.vector.tensor_tensor(
                        w1[:, s], w1[:, s], w1[:, s], Alu.mult)

            # per-round tail so it pipelines with this tile's matmuls:
            # mask = (max(a0,a1,a2) <= 1), count on accum_out;
            # n2 sum; mm = mask*n2; Sqrt accum -> per-round partials
            mx = work.tile([128, N_PTS], f16, tag="mx")
            mask = work.tile([128, N_PTS], f16, tag="mask")
            rcv = small.tile([128, 4], f32, tag="rcv")
            rsv = small.tile([128, 4], f32, tag="rsv")
            for j in range(N_CHUNKS // 2):
                cs = slice(j * 2 * CHUNK, (j + 1) * 2 * CHUNK)
                nc.vector.tensor_tensor(
                    mx[:, cs], a0[:, cs], a1[:, cs], Alu.max)
                nc.vector.tensor_tensor(
                    mx[:, cs], mx[:, cs], a2[:, cs], Alu.max)
                nc.vector.tensor_scalar(
                    mask[:, cs], mx[:, cs], 1.0, 0.0, Alu.is_le, Alu.add,
                    accum_out=rcv[:, j:j + 1])
                nc.vector.tensor_tensor(
                    w0[:, cs], w0[:, cs], w1[:, cs], Alu.add)
                nc.vector.tensor_tensor(
                    w0[:, cs], w0[:, cs], w2[:, cs], Alu.add)
                nc.vector.tensor_tensor(
                    mx[:, cs], mask[:, cs], w0[:, cs], Alu.mult)
                nc.scalar.activation(
                    w1[:, cs], mx[:, cs], Act.Sqrt,
                    accum_out=rsv[:, j:j + 1])
            rc = small.tile([128, 1], f32, tag="rc")
            rs = small.tile([128, 1], f32, tag="rs")
            nc.vector.tensor_reduce(
                rc[:], rcv[:], mybir.AxisListType.X, Alu.add)
            nc.vector.tensor_reduce(
                rs[:], rsv[:], mybir.AxisListType.X, Alu.add)

            # out = (cnt==0)*10000 + (-10000*rs)/max(cnt,1)
            rc1 = small.tile([128, 1], f32, tag="rc1")
            nc.vector.tensor_scalar(rc1[:], rc[:], 1.0, None, Alu.max)
            inv = small.tile([128, 1], f32, tag="inv")
            nc.vector.reciprocal(inv[:], rc1[:])
            val = small.tile([128, 1], f32, tag="val")
            nc.vector.scalar_tensor_tensor(
                val[:], rs[:], -DIST_COEFF, inv[:], Alu.mult, Alu.mult)
            zer = small.tile([128, 1], f32, tag="zer")
            nc.vector.tensor_scalar(zer[:], rc[:], 0.0, None, Alu.is_le)
            ot = small.tile([128, 1], f32, tag="ot")
            nc.vector.scalar_tensor_tensor(
                ot[:], zer[:], DIST_COEFF, val[:], Alu.mult, Alu.add)
            nc.sync.dma_start(out=out_ap[r, :], in_=ot[:])
    nc.compile()
    return nc


def _get_nc():
    if "nc" not in _NC_CACHE:
        _NC_CACHE["nc"] = _build_bass()
    return _NC_CACHE["nc"]


def _host_coefficients(trans, quat):
    """Per-batch A [B,3,3], o [B,3], g [B,3] in f32 (computed in f64)."""
    q = np.asarray(quat, np.float64)
    t = np.asarray(trans, np.float64)
    B = q.shape[0]
    s = (q * q).sum(-1)
    qi = np.concatenate([-q[:, :3], q[:, 3:]], -1) / s[:, None]
    v, w = qi[:, :3], qi[:, 3]
    vv = v[:, :, None] * v[:, None, :]
    w2mv = w * w - (v * v).sum(-1)
    Vx = np.zeros((B, 3, 3))
    Vx[:, 0, 1] = -v[:, 2]
    Vx[:, 0, 2] = v[:, 1]
    Vx[:, 1, 0] = v[:, 2]
    Vx[:, 1, 2] = -v[:, 0]
    Vx[:, 2, 0] = -v[:, 1]
    Vx[:, 2, 1] = v[:, 0]
    R = (w2mv[:, None, None] * np.eye(3)
         + 2.0 * vv
         + 2.0 * w[:, None, None] * Vx)
    A = R / HALF_EXT[None, :, None]
    o = -(t + BOX_CENTER[None, :]) / HALF_EXT[None, :]
    g = np.einsum("bij,bj->bi", R.transpose(0, 2, 1), t)
    return A.astype(np.float32), o.astype(np.float32), g.astype(np.float32)


def _make_in_maps(trans, quat, pc):
    A, o, g = _host_coefficients(trans, quat)
    coef_full = np.concatenate(
        [o, -g, np.zeros((B_FULL, 2), np.float32)], axis=1)  # [B,8]
    # planar fp16 [3, B, N]
    pcT = np.ascontiguousarray(
        np.asarray(pc, np.float32).transpose(2, 0, 1)).astype(np.float16)
    # diagonal weights [tile, i, c] per core
    idx = np.arange(128)
    in_maps = []
    for cidx in range(N_CORES):
        bs, be = cidx * B_CORE, (cidx + 1) * B_CORE
        Ac = A[bs:be].reshape(N_TILES, 128, 3, 3)
        wdc = np.zeros((N_TILES, 3, 3, 128, 128), np.float16)
        wdc[:, :, :, idx, idx] = np.transpose(
            Ac, (0, 2, 3, 1)).astype(np.float16)
        in_maps.append({
            "xyz": np.ascontiguousarray(pcT[:, bs:be, :]),
            "coef": np.ascontiguousarray(
                coef_full[bs:be].reshape(N_TILES, 128, 8)),
            "wd": np.ascontiguousarray(
                np.transpose(wdc.reshape(N_TILES, 9, 128, 128),
                             (0, 2, 1, 3)).reshape(N_TILES, 128, 9 * 128)),
        })
    return in_maps


def run_spmd(trans, quat, pc, **spmd_kwargs):
    """Shard, run on 8 cores, gather. Returns (output, BassKernelResults)."""
    in_maps = _make_in_maps(trans, quat, pc)
    res = run_bass_kernel_spmd(
        _get_nc(), in_maps, list(range(N_CORES)), **spmd_kwargs)
    outs = [res.results[i]["out"] for i in range(N_CORES)]
    full = np.concatenate(outs, axis=0).astype(np.float32)
    return full, res


def kernel(trans, quat, pc):
    full, _ = run_spmd(trans, quat, pc)
    return full


# revision 40
# speedup vs baseline: 1.0185x; 1.0050x over previous
"""Trainium2 Bass kernel for CollisionDistanceEvaluator (segment_reduce).

Contract: kernel(**inputs) takes FULL inputs (trans [4096,3] f32,
quat [4096,4] f32, pc [4096,4096,3] f32) and returns the FULL output
[4096,1] f32, running the heavy per-point work on 8 NeuronCores
(pure data-parallel over the batch dim, 512 batches/core).

Math: reference rotates pc by inv(quat) (unit norm -> pure rotation R),
translates by -trans, tests an axis-aligned box, and takes the
per-batch masked mean of point norms. Host precomputes per batch:
    A[b] = R[b] / H[:,None]      (H = box half extents)
    o[b] = -(trans[b] + C) / H   (C = box center)
    g[b] = R[b]^T trans[b]       (|R p - t| = |p - g|, R orthogonal)
Device, per point p:
    u'_i  = A_i . p              <- TENSOR engine: 3 accumulated matmuls
                                    with diagonal [128,128] weights
                                    (per-batch scale), batch=partitions
    inside = max_i |u'_i + o_i| <= 1
    n2    = sum_i (p_i - g_i)^2
    out[b] = -10000 * sum(inside*sqrt(n2))/max(cnt,1)  (or +10000 if cnt==0)

Engine split per 128-batch tile (batch->partitions, 4096 points->free):
  PE   : 18 diag matmuls per 1024-pt round (u' planes, fp32 PSUM,
         2 banks per plane; ACT consumes both banks in one op)
  ACT  : Abs(u_i'+o_i) from PSUM; Square(x-g0) from SBUF;
         Sqrt(mask*n2) with free row-sum accumulation
  DVE  : max-combine, is_le mask with count on accum_out,
         (y-g1)^2 and (z-g2)^2 squares, n2 adds, mask*n2
"""

import numpy as np

import concourse.bass as bass
import concourse.bacc as bacc
import concourse.mybir as mybir
from concourse.tile import TileContext
from concourse.bass_utils import run_bass_kernel_spmd

def _ensure_ntff_hook():
    """Register the axon NTFF profile hook if the image's antenv lacks it."""
    import sys
    import types
    try:
        from antenv.axon_hooks import get_axon_ntff_profile_hook  # noqa
        return
    except ImportError:
        pass
    try:
        import antenv
        from trn_agent_boot.trn_boot import _ntff_profile_via_ctypes
        mod = types.ModuleType("antenv.axon_hooks")
        mod._hook = _ntff_profile_via_ctypes("/opt/axon/libaxon_pjrt.so")

        def set_axon_ntff_profile_hook(h):
            mod._hook = h

        def get_axon_ntff_profile_hook():
            return mod._hook

        mod.set_axon_ntff_profile_hook = set_axon_ntff_profile_hook
        mod.get_axon_ntff_profile_hook = get_axon_ntff_profile_hook
        sys.modules["antenv.axon_hooks"] = mod
        antenv.axon_hooks = mod
    except Exception:
        pass


_ensure_ntff_hook()

N_CORES = 8
B_FULL, N_PTS = 4096, 4096
B_CORE = B_FULL // N_CORES          # 512
N_TILES = B_CORE // 128             # 4
CHUNK = 512                          # PSUM bank: 512 f32
N_CHUNKS = N_PTS // CHUNK           # 8

DIST_THRESHOLD = 0.001
DIST_COEFF = 10000.0
BOX_CENTER = np.array([-0.001782, 1.005e-05, 0.0431621], dtype=np.float64)
HALF_EXT = np.array([
    0.204416 / 2 + DIST_THRESHOLD,
    0.0632517 / 2 + DIST_THRESHOLD,
    0.1381738 / 2 + DIST_THRESHOLD,
], dtype=np.float64)

_NC_CACHE = {}


def _build_bass():
    f16, f32 = mybir.dt.float16, mybir.dt.float32
    Alu = mybir.AluOpType
    Act = mybir.ActivationFunctionType

    nc = bacc.Bacc()
    xyz = nc.declare_dram_parameter(
        "xyz", [3, B_CORE, N_PTS], f16, isOutput=False)
    # per-batch scalars: cols 0-2 = o_i, cols 3-5 = -g_i
    coef = nc.declare_dram_parameter(
        "coef", [N_TILES, 128, 8], f32, isOutput=False)
    # diagonal PE weights: [tile, i, c] -> diag(A[batch, i, c])
    wd = nc.declare_dram_parameter(
        "wd", [N_TILES, 128, 9 * 128], f16, isOutput=False)
    out = nc.declare_dram_parameter("out", [B_CORE, 1], f32, isOutput=True)
    xyz_ap, coef_ap, wd_ap, out_ap = xyz[:], coef[:], wd[:], out[:]

    with TileContext(nc) as tc, \
            tc.tile_pool(name="data", bufs=2) as data, \
            tc.tile_pool(name="mask", bufs=2) as maskp, \
            tc.tile_pool(name="sq", bufs=2) as sqp, \
            tc.tile_pool(name="work", bufs=2) as work, \
            tc.tile_pool(name="wpool", bufs=2) as wpool, \
            tc.tile_pool(name="psum", bufs=1, space="PSUM") as psum, \
            tc.tile_pool(name="small", bufs=4) as small:
        # PE warm-up: junk matmuls during the initial DMA ramp keep the
        # HAM clock gate open so the first real matmuls run at 2.4 GHz
        wu = wpool.tile([128, 128], f16, tag="warm")
        nc.vector.memset(wu[:], 0.0)
        pwu = psum.tile([128, CHUNK], f32, tag="warm")
        for k in range(96):
            nc.tensor.matmul(pwu[:, :128], wu[:], wu[:, :],
                             start=True, stop=True)

        for t in range(N_TILES):
            r = slice(t * 128, (t + 1) * 128)
            # dispatch order = need order: coef + first-half planes
            # gate the first compute; weights are needed a bit later
            ct = small.tile([128, 8], f32, tag="coef")
            nc.sync.dma_start(out=ct[:], in_=coef_ap[t, :, :])
            xt = data.tile([128, N_PTS], f16, tag="x")
            yt = data.tile([128, N_PTS], f16, tag="y")
            zt = data.tile([128, N_PTS], f16, tag="z")
            half = N_PTS // 2
            h0 = slice(0, half)
            nc.sync.dma_start(out=zt[:, h0], in_=xyz_ap[2, r, h0])
            nc.sync.dma_start(out=xt[:, h0], in_=xyz_ap[0, r, h0])
            nc.sync.dma_start(out=yt[:, h0], in_=xyz_ap[1, r, h0])
            # per-tile weights (one DMA), double-buffered for prefetch
            wt = wpool.tile([128, 9 * 128], f16, tag="wt")
            nc.sync.dma_start(out=wt[:], in_=wd_ap[t, :, :])
            h1 = slice(half, N_PTS)
            nc.sync.dma_start(out=zt[:, h1], in_=xyz_ap[2, r, h1])
            nc.sync.dma_start(out=xt[:, h1], in_=xyz_ap[0, r, h1])
            nc.sync.dma_start(out=yt[:, h1], in_=xyz_ap[1, r, h1])
            planes = (xt, yt, zt)

            # mask-plane tiles (full 4096 wide, filled chunkwise)
            a0 = maskp.tile([128, N_PTS], f16, tag="a0")
            a1 = maskp.tile([128, N_PTS], f16, tag="a1")
            a2 = maskp.tile([128, N_PTS], f16, tag="a2")

            def wsel(i, c, wt=wt):
                w = 3 * i + c
                return wt[:, w * 128:(w + 1) * 128]

            w0 = sqp.tile([128, N_PTS], f16, tag="w0")
            w1 = sqp.tile([128, N_PTS], f16, tag="w1")
            w2 = sqp.tile([128, N_PTS], f16, tag="w2")
            # rounds of 2 PSUM banks (1024 pts): one matmul writes <= 1
            # bank, but ACT reads across both in a single op
            for j in range(N_CHUNKS // 2):
                cs = slice(j * 2 * CHUNK, (j + 1) * 2 * CHUNK)
                u0 = psum.tile([128, 2 * CHUNK], f32, tag="u0")
                u1 = psum.tile([128, 2 * CHUNK], f32, tag="u1")
                u2 = psum.tile([128, 2 * CHUNK], f32, tag="u2")
                for i, ut in enumerate((u0, u1, u2)):
                    for c in range(3):
                        for h in range(2):
                            hs = slice(h * CHUNK, (h + 1) * CHUNK)
                            ps = slice((j * 2 + h) * CHUNK,
                                       (j * 2 + h + 1) * CHUNK)
                            nc.tensor.matmul(
                                ut[:, hs], wsel(i, c), planes[c][:, ps],
                                start=(c == 0), stop=(c == 2))
                # |u'+o| on ACT (fast PSUM reads, free bias add)
                nc.scalar.activation(
                    a0[:, cs], u0[:], Act.Abs, bias=ct[:, 0:1])
                nc.scalar.activation(
                    a1[:, cs], u1[:], Act.Abs, bias=ct[:, 1:2])
                nc.scalar.activation(
                    a2[:, cs], u2[:], Act.Abs, bias=ct[:, 2:3])
                # interleave ACT square pieces between ABS bursts so
                # the n2 adds never wait on a late x/y square
                if j < 2:
                    s = slice(j * half, (j + 1) * half)
                    nc.scalar.activation(
                        w0[:, s], xt[:, s], Act.Square, bias=ct[:, 3:4])
                elif j == 2:
                    s = slice(half, N_PTS)
                    nc.scalar.activation(
                        w1[:, s], yt[:, s], Act.Square, bias=ct[:, 4:5])

            # remaining squares on DVE: (y-g1)^2 first half, (z-g2)^2
            # halved so work starts as soon as the first DMAs land.
            for hh in range(2):
                s = slice(hh * half, (hh + 1) * half)
                nc.vector.tensor_scalar(
                    w2[:, s], zt[:, s], ct[:, 5:6], None, Alu.add)
                nc.vector.tensor_tensor(
                    w2[:, s], w2[:, s], w2[:, s], Alu.mult)
                if hh == 0:
                    nc.vector.tensor_scalar(
                        w1[:, s], yt[:, s], ct[:, 4:5], None, Alu.add)
                    nc.vector.tensor_tensor(
                        w1[:, s], w1[:, s], w1[:, s], Alu.mult)

            # per-round tail so it pipelines with this tile's matmuls:
            # mask = (max(a0,a1,a2) <= 1), count on accum_out;
            # n2 sum; mm = mask*n2; Sqrt accum -> per-round partials
            mx = work.tile([128, N_PTS], f16, tag="mx")
            mask = work.tile([128, N_PTS], f16, tag="mask")
            rcv = small.tile([128, 4], f32, tag="rcv")
            rsv = small.tile([128, 4], f32, tag="rsv")
            for j in range(N_CHUNKS // 2):
                cs = slice(j * 2 * CHUNK, (j + 1) * 2 * CHUNK)
                nc.vector.tensor_tensor(
                    mx[:, cs], a0[:, cs], a1[:, cs], Alu.max)
                nc.vector.tensor_tensor(
                    mx[:, cs], mx[:, cs], a2[:, cs], Alu.max)
                nc.vector.tensor_scalar(
                    mask[:, cs], mx[:, cs], 1.0, 0.0, Alu.is_le, Alu.add,
                    accum_out=rcv[:, j:j + 1])
                nc.vector.tensor_tensor(
                    w0[:, cs], w0[:, cs], w1[:, cs], Alu.add)
                nc.vector.tensor_tensor(
                    w0[:, cs], w0[:, cs], w2[:, cs], Alu.add)
                nc.vector.tensor_tensor(
                    mx[:, cs], mask[:, cs], w0[:, cs], Alu.mult)
                nc.scalar.activation(
                    w1[:, cs], mx[:, cs], Act.Sqrt,
                    accum_out=rsv[:, j:j + 1])
            rc = small.tile([128, 1], f32, tag="rc")
            rs = small.tile([128, 1], f32, tag="rs")
            nc.vector.tensor_reduce(
                rc[:], rcv[:], mybir.AxisListType.X, Alu.add)
            nc.vector.tensor_reduce(
                rs[:], rsv[:], mybir.AxisListType.X, Alu.add)

            # out = (cnt==0)*10000 + (-10000*rs)/max(cnt,1)
            rc1 = small.tile([128, 1], f32, tag="rc1")
            nc.vector.tensor_scalar(rc1[:], rc[:], 1.0, None, Alu.max)
            inv = small.tile([128, 1], f32, tag="inv")
            nc.vector.reciprocal(inv[:], rc1[:])
            val = small.tile([128, 1], f32, tag="val")
            nc.vector.scalar_tensor_tensor(
                val[:], rs[:], -DIST_COEFF, inv[:], Alu.mult, Alu.mult)
            zer = small.tile([128, 1], f32, tag="zer")
            nc.vector.tensor_scalar(zer[:], rc[:], 0.0, None, Alu.is_le)
            ot = small.tile([128, 1], f32, tag="ot")
            nc.vector.scalar_tensor_tensor(
                ot[:], zer[:], DIST_COEFF, val[:], Alu.mult, Alu.add)
            nc.sync.dma_start(out=out_ap[r, :], in_=ot[:])
    nc.compile()
    return nc


def _get_nc():
    if "nc" not in _NC_CACHE:
        _NC_CACHE["nc"] = _build_bass()
    return _NC_CACHE["nc"]


def _host_coefficients(trans, quat):
    """Per-batch A [B,3,3], o [B,3], g [B,3] in f32 (computed in f64)."""
    q = np.asarray(quat, np.float64)
    t = np.asarray(trans, np.float64)
    B = q.shape[0]
    s = (q * q).sum(-1)
    qi = np.concatenate([-q[:, :3], q[:, 3:]], -1) / s[:, None]
    v, w = qi[:, :3], qi[:, 3]
    vv = v[:, :, None] * v[:, None, :]
    w2mv = w * w - (v * v).sum(-1)
    Vx = np.zeros((B, 3, 3))
    Vx[:, 0, 1] = -v[:, 2]
    Vx[:, 0, 2] = v[:, 1]
    Vx[:, 1, 0] = v[:, 2]
    Vx[:, 1, 2] = -v[:, 0]
    Vx[:, 2, 0] = -v[:, 1]
    Vx[:, 2, 1] = v[:, 0]
    R = (w2mv[:, None, None] * np.eye(3)
         + 2.0 * vv
         + 2.0 * w[:, None, None] * Vx)
    A = R / HALF_EXT[None, :, None]
    o = -(t + BOX_CENTER[None, :]) / HALF_EXT[None, :]
    g = np.einsum("bij,bj->bi", R.transpose(0, 2, 1), t)
    return A.astype(np.float32), o.astype(np.float32), g.astype(np.float32)


def _make_in_maps(trans, quat, pc):
    A, o, g = _host_coefficients(trans, quat)
    coef_full = np.concatenate(
        [o, -g, np.zeros((B_FULL, 2), np.float32)], axis=1)  # [B,8]
    # planar fp16 [3, B, N]
    pcT = np.ascontiguousarray(
        np.asarray(pc, np.float32).transpose(2, 0, 1)).astype(np.float16)
    # diagonal weights [tile, i, c] per core
    idx = np.arange(128)
    in_maps = []
    for cidx in range(N_CORES):
        bs, be = cidx * B_CORE, (cidx + 1) * B_CORE
        Ac = A[bs:be].reshape(N_TILES, 128, 3, 3)
        wdc = np.zeros((N_TILES, 3, 3, 128, 128), np.float16)
        wdc[:, :, :, idx, idx] = np.transpose(
            Ac, (0, 2, 3, 1)).astype(np.float16)
        in_maps.append({
            "xyz": np.ascontiguousarray(pcT[:, bs:be, :]),
            "coef": np.ascontiguousarray(
                coef_full[bs:be].reshape(N_TILES, 128, 8)),
            "wd": np.ascontiguousarray(
                np.transpose(wdc.reshape(N_TILES, 9, 128, 128),
                             (0, 2, 1, 3)).reshape(N_TILES, 128, 9 * 128)),
        })
    return in_maps


def run_spmd(trans, quat, pc, **spmd_kwargs):
    """Shard, run on 8 cores, gather. Returns (output, BassKernelResults)."""
    in_maps = _make_in_maps(trans, quat, pc)
    res = run_bass_kernel_spmd(
        _get_nc(), in_maps, list(range(N_CORES)), **spmd_kwargs)
    outs = [res.results[i]["out"] for i in range(N_CORES)]
    full = np.concatenate(outs, axis=0).astype(np.float32)
    return full, res


def kernel(trans, quat, pc):
    full, _ = run_spmd(trans, quat, pc)
    return full
